# revision 4
# baseline (speedup 1.0000x reference)
"""ChamferLoss Trainium2 kernel (8 NeuronCores, bass/Tile).

pred, target: [2, 16384, 3] fp32 -> scalar fp32
  d[b,n,m] = ||pred[b,n] - target[b,m]||
  out = mean(min_m d) + mean(min_n d)

Sharding: core c = (batch b=c//4, pred-quarter q=c%4): 4096 preds x all 16384
targets. Per core:
  - PE: d^2 tiles via one K=128 bf16 matmul. The 13 augmented contraction
    rows (bf16 hi/lo split of the coordinates plus squared-norm terms, so
    d^2 = p2 + t2 - 2 p.t accumulates in fp32 PSUM with ~1e-4 abs error) are
    replicated 9x (117 rows + 11 zero rows): a K=13 matmul leaves 90% of the
    PE array idle and the HAM clock gate then never lifts the 1.2 GHz cold
    throttle; at K=128 the array is ~91% active, runs at 2.4 GHz, and the
    9x-scaled sum is undone for free by the ScalarE convert's scale=1/9.
  - ScalarE: PSUM fp32 -> SBUF fp16 conversion (x 1/9).
  - VectorE: fp16 running minima (2x perf mode): forward accumulator per
    128-pred block folded by a log2 tensor_tensor tree, backward accumulator
    [128, 16384] across pred blocks.
  - Tail: PE transposes the backward accumulator in 128-col chunks (4 chunks
    per PSUM tile); VectorE reduce_min over [128, 4, 128] -> per-target min.
Host: concatenates forward mins, elementwise-mins backward partials across
quarters, then sqrt + means (O(N) work).
"""

import ml_dtypes
import numpy as np

import concourse.bass as bass
import concourse.tile as tile
from concourse import mybir

F32 = mybir.dt.float32
F16 = mybir.dt.float16
BF16 = mybir.dt.bfloat16

B = 2
N = 16384          # preds per batch
M = 16384          # targets per batch
NQ = N // 4        # preds per core
KA = 13            # base augmented contraction depth
NREP = 9           # replication count (13*9 = 117 <= 128)
K = 128            # padded contraction depth
G = 2048           # PSUM group width (4 banks)
NB = NQ // 128     # pred blocks per core (32)
NG = M // G        # target groups (8)
MM_N = 512         # matmul free dim (one PSUM bank)
N_CORES = 8


# --------------------------------------------------------------------------
# Workaround: this walrus build accepts at most one sync-wait command per
# instruction. Hoist extra waits onto same-engine NoOps placed just before.
# --------------------------------------------------------------------------

def _split_sync_waits(nc):
    counter = 0
    for block in nc.m.functions[0].blocks:
        insts = block.instructions
        out = []
        changed = False
        for inst in insts:
            si = inst.sync_info
            if si is not None and si.on_wait and len(si.on_wait) > 1:
                waits = list(si.on_wait)
                for w in waits[:-1]:
                    counter += 1
                    out.append(
                        mybir.InstNoOp(
                            name=f"waitnop-{counter}",
                            engine=inst.engine,
                            sync_info=mybir.SyncInfo(on_wait=[w], on_update=[]),
                        )
                    )
                si.on_wait = waits[-1:]
                changed = True
            out.append(inst)
        if changed:
            block.instructions = out


def _patch_bass():
    if getattr(bass.Bass, "_split_waits_patched", False):
        return
    orig = bass.Bass.to_json_bytes

    def to_json_bytes(self, *a, **kw):
        _split_sync_waits(self)
        return orig(self, *a, **kw)

    bass.Bass.to_json_bytes = to_json_bytes
    bass.Bass._split_waits_patched = True


# --------------------------------------------------------------------------
# Kernel builder
# --------------------------------------------------------------------------

def build_kernel(n_loop: int = 0):
    """n_loop=0: production straight-line kernel. n_loop>0: wrap the main
    (idempotent) compute in a For_i loop for slope timing."""
    _patch_bass()
    nc = bass.Bass()
    paug_d = nc.dram_tensor("paug", [K, NQ], BF16, kind="ExternalInput")
    taug_d = nc.dram_tensor("taug", [K, M], BF16, kind="ExternalInput")
    fmin_d = nc.dram_tensor("fmin", [128, NB], F32, kind="ExternalOutput")
    bmin_d = nc.dram_tensor("bmin", [128, M // 128], F32, kind="ExternalOutput")

    with tile.TileContext(nc) as tc:
        with (
            tc.tile_pool(name="singles", bufs=1) as singles,
            tc.tile_pool(name="work", bufs=2) as work,
        ):
            paug = singles.tile([K, NQ], BF16)
            taug = singles.tile([K, M], BF16)
            bacc = singles.tile([128, M], F16)
            fmin_sb = singles.tile([128, NB], F32)
            bmin_sb = singles.tile([128, M // 128], F32)

            nc.sync.dma_start(out=paug[:], in_=paug_d[:])
            for g in range(NG):
                nc.sync.dma_start(
                    out=taug[:, g * G:(g + 1) * G],
                    in_=taug_d[:, g * G:(g + 1) * G],
                )

            ident = singles.tile([128, 128], F16)
            nc.gpsimd.memset(ident[:], 0.0)
            nc.gpsimd.affine_select(
                out=ident[:],
                in_=ident[:],
                compare_op=mybir.AluOpType.not_equal,
                fill=1.0,
                base=0,
                pattern=[[-1, 128]],
                channel_multiplier=1,
            )

            CVT_SCALE = 1.0 / NREP

            def main_compute():
                for nb in range(NB):
                    lhsT = paug[:, nb * 128:(nb + 1) * 128]
                    facc = work.tile([128, G], F16, name=f"facc{nb}",
                                     tag="facc")
                    for g in range(NG):
                        d2 = psum.tile([128, G], F32, name=f"d2_{nb}_{g}",
                                       tag="d2")
                        for j in range(G // MM_N):
                            nc.tensor.matmul(
                                d2[:, j * MM_N:(j + 1) * MM_N],
                                lhsT,
                                taug[:, g * G + j * MM_N:
                                     g * G + (j + 1) * MM_N],
                                start=True,
                                stop=True,
                            )
                        bslice = bacc[:, g * G:(g + 1) * G]
                        if nb == 0:
                            # first pred block initializes the backward acc
                            nc.scalar.activation(
                                out=bslice, in_=d2[:],
                                func=mybir.ActivationFunctionType.Copy,
                                scale=CVT_SCALE,
                            )
                            if g == 0:
                                nc.vector.tensor_copy(facc[:], bslice)
                            else:
                                nc.vector.tensor_tensor(
                                    out=facc[:], in0=facc[:], in1=bslice,
                                    op=mybir.AluOpType.min,
                                )
                        elif g == 0:
                            # convert straight into the fresh forward acc
                            nc.scalar.activation(
                                out=facc[:], in_=d2[:],
                                func=mybir.ActivationFunctionType.Copy,
                                scale=CVT_SCALE,
                            )
                            nc.vector.tensor_tensor(
                                out=bslice, in0=bslice, in1=facc[:],
                                op=mybir.AluOpType.min,
                            )
                        else:
                            cvt = work.tile([128, G], F16,
                                            name=f"cvt{nb}_{g}", tag="cvt")
                            nc.scalar.activation(
                                out=cvt[:], in_=d2[:],
                                func=mybir.ActivationFunctionType.Copy,
                                scale=CVT_SCALE,
                            )
                            nc.vector.tensor_tensor(
                                out=bslice, in0=bslice, in1=cvt[:],
                                op=mybir.AluOpType.min,
                            )
                            nc.vector.tensor_tensor(
                                out=facc[:], in0=facc[:], in1=cvt[:],
                                op=mybir.AluOpType.min,
                            )
                    # log2 tree fold of facc -> fmin column
                    w = G // 2
                    while w >= 2:
                        nc.vector.tensor_tensor(
                            out=facc[:, 0:w], in0=facc[:, 0:w],
                            in1=facc[:, w:2 * w], op=mybir.AluOpType.min,
                        )
                        w //= 2
                    nc.vector.tensor_tensor(
                        out=fmin_sb[:, nb:nb + 1], in0=facc[:, 0:1],
                        in1=facc[:, 1:2], op=mybir.AluOpType.min,
                    )

            with tc.tile_pool(name="psum", bufs=2, space="PSUM") as psum:
                def tail_fold():
                    # backward partition fold: transpose 4x128 cols per PSUM
                    # tile, one reduce per 4 chunks
                    for t4 in range(M // 512):
                        tp = psum.tile([128, 512], F16, name=f"tp{t4}",
                                       tag="d2")
                        for u in range(4):
                            t = t4 * 4 + u
                            nc.tensor.transpose(
                                tp[:, u * 128:(u + 1) * 128],
                                bacc[:, t * 128:(t + 1) * 128],
                                ident[:],
                            )
                        nc.vector.tensor_reduce(
                            out=bmin_sb[:, t4 * 4:(t4 + 1) * 4],
                            in_=tp[:].rearrange("p (u f) -> p u f", u=4),
                            axis=mybir.AxisListType.X,
                            op=mybir.AluOpType.min,
                        )

                if n_loop:
                    with tc.For_i(0, n_loop, 1):
                        main_compute()
                        tail_fold()
                else:
                    main_compute()
                    tail_fold()

            nc.sync.dma_start(out=fmin_d[:], in_=fmin_sb[:])
            nc.sync.dma_start(out=bmin_d[:], in_=bmin_sb[:])
    return nc


# --------------------------------------------------------------------------
# Host-side prep: augmented coordinate matrices (bf16 hi/lo split), the
# 13-row base replicated NREP times and zero-padded to K=128 rows.
#   k0-2:  pred -2*hi | target hi
#   k3-5:  pred -2*hi | target lo
#   k6-8:  pred -2*lo | target hi
#   k9:    pred 1     | target t2_hi
#   k10:   pred 1     | target t2_lo
#   k11:   pred p2_hi | target 1
#   k12:   pred p2_lo | target 1
# --------------------------------------------------------------------------

def _bf16(x):
    return x.astype(ml_dtypes.bfloat16)


def _aug_parts(coords):
    c = coords.astype(np.float32).T  # [3, n]
    hi = _bf16(c)
    lo = _bf16(c - hi.astype(np.float32))
    n2 = np.sum(c * c, axis=0, dtype=np.float32)
    n2_hi = _bf16(n2)
    n2_lo = _bf16(n2 - n2_hi.astype(np.float32))
    return hi, lo, n2_hi, n2_lo


def _replicate(base):
    out = np.zeros((K, base.shape[1]), dtype=ml_dtypes.bfloat16)
    for r in range(NREP):
        out[r * KA:(r + 1) * KA] = base
    return out


def _aug_pred(coords):
    hi, lo, n2_hi, n2_lo = _aug_parts(coords)
    base = np.zeros((KA, coords.shape[0]), dtype=ml_dtypes.bfloat16)
    m2hi = _bf16(-2.0 * hi.astype(np.float32))
    m2lo = _bf16(-2.0 * lo.astype(np.float32))
    base[0:3] = m2hi
    base[3:6] = m2hi
    base[6:9] = m2lo
    base[9] = 1.0
    base[10] = 1.0
    base[11] = n2_hi
    base[12] = n2_lo
    return _replicate(base)


def _aug_target(coords):
    hi, lo, n2_hi, n2_lo = _aug_parts(coords)
    base = np.zeros((KA, coords.shape[0]), dtype=ml_dtypes.bfloat16)
    base[0:3] = hi
    base[3:6] = lo
    base[6:9] = hi
    base[9] = n2_hi
    base[10] = n2_lo
    base[11] = 1.0
    base[12] = 1.0
    return _replicate(base)


def make_in_maps(pred, target):
    pred = np.asarray(pred, dtype=np.float32)
    target = np.asarray(target, dtype=np.float32)
    in_maps = []
    taugs = [_aug_target(target[b]) for b in range(B)]
    for c in range(N_CORES):
        b, q = divmod(c, 4)
        in_maps.append({
            "paug": _aug_pred(pred[b, q * NQ:(q + 1) * NQ]),
            "taug": taugs[b],
        })
    return in_maps


def postprocess(results):
    total = np.float64(0.0)
    for b in range(B):
        fwd = []
        bwd = None
        for q in range(4):
            r = results[b * 4 + q]
            fwd.append(np.asarray(r["fmin"]).T.reshape(-1))   # n = nb*128+p
            bm = np.asarray(r["bmin"]).T.reshape(-1)          # m = t*128+p
            bwd = bm if bwd is None else np.minimum(bwd, bm)
        fwd = np.concatenate(fwd)
        f = np.sqrt(np.maximum(fwd, 0.0, dtype=np.float32)).mean(dtype=np.float64)
        g = np.sqrt(np.maximum(bwd, 0.0, dtype=np.float32)).mean(dtype=np.float64)
        total += (f + g) / B
    return np.asarray(total, dtype=np.float32)


# --------------------------------------------------------------------------
# PJRT runner (jit built once per process)
# --------------------------------------------------------------------------

def make_runner(nc, n_cores=N_CORES):
    import jax
    from jax.sharding import Mesh, PartitionSpec
    from jax.experimental.shard_map import shard_map
    from concourse.bass2jax import (
        _bass_exec_p,
        install_neuronx_cc_hook,
        partition_id_tensor,
    )

    install_neuronx_cc_hook()
    partition_name = (
        nc.partition_id_tensor.name if nc.partition_id_tensor else None
    )

    in_names, out_names, out_avals, zero_outs = [], [], [], []
    for alloc in nc.m.functions[0].allocations:
        if not isinstance(alloc, mybir.MemoryLocationSet):
            continue
        name = alloc.memorylocations[0].name
        if alloc.kind == "ExternalInput":
            if name != partition_name:
                in_names.append(name)
        elif alloc.kind == "ExternalOutput":
            shape = tuple(alloc.tensor_shape)
            dtype = mybir.dt.np(alloc.dtype)
            out_names.append(name)
            out_avals.append(jax.core.ShapedArray(shape, dtype))
            zero_outs.append(np.zeros(shape, dtype))
    n_params = len(in_names)
    all_in_names = list(in_names) + list(out_names)
    if partition_name is not None:
        all_in_names.append(partition_name)

    def _body(*args):
        operands = list(args)
        if partition_name is not None:
            operands.append(partition_id_tensor())
        outs = _bass_exec_p.bind(
            *operands,
            out_avals=tuple(out_avals),
            in_names=tuple(all_in_names),
            out_names=tuple(out_names),
            lowering_input_output_aliases=(),
            sim_require_finite=True,
            sim_require_nnan=True,
            nc=nc,
        )
        return tuple(outs)

    devices = jax.devices()[:n_cores]
    mesh = Mesh(np.asarray(devices), ("core",))
    in_specs = (PartitionSpec("core"),) * (n_params + len(out_names))
    out_specs = (PartitionSpec("core"),) * len(out_names)
    jitted = jax.jit(
        shard_map(_body, mesh=mesh, in_specs=in_specs, out_specs=out_specs,
                  check_rep=False),
        keep_unused=True,
    )

    def run(in_maps):
        import jax as _jax
        concat_in = [
            np.concatenate(
                [np.asarray(in_maps[c][n]) for c in range(n_cores)], axis=0
            )
            for n in in_names
        ]
        concat_zeros = [
            np.zeros((n_cores * z.shape[0], *z.shape[1:]), z.dtype)
            for z in zero_outs
        ]
        outs = jitted(*concat_in, *concat_zeros)
        _jax.block_until_ready(outs)
        return [
            {
                name: np.asarray(outs[i]).reshape(
                    n_cores, *out_avals[i].shape
                )[c]
                for i, name in enumerate(out_names)
            }
            for c in range(n_cores)
        ]

    return run


_CACHE = {}


def kernel(pred, target):
    if "run" not in _CACHE:
        _CACHE["run"] = make_runner(build_kernel(0))
    results = _CACHE["run"](make_in_maps(pred, target))
    return postprocess(results)


# revision 7
# speedup vs baseline: 1.6699x; 1.6699x over previous
"""ChamferLoss Trainium2 kernel (8 NeuronCores, bass/Tile).

pred, target: [2, 16384, 3] fp32 -> scalar fp32
  d[b,n,m] = ||pred[b,n] - target[b,m]||
  out = mean(min_m d) + mean(min_n d)

Sharding: core c = (batch b=c//4, pred-quarter q=c%4): 4096 preds x all 16384
targets. Per core:
  - PE: d^2 tiles via one K=128 bf16 matmul. The 13 augmented contraction
    rows (bf16 hi/lo split of the coordinates plus squared-norm terms, so
    d^2 = p2 + t2 - 2 p.t accumulates in fp32 PSUM with ~1e-4 abs error) are
    replicated 9x (117 rows + 11 zero rows): a K=13 matmul leaves 90% of the
    PE array idle and the HAM clock gate then never lifts the 1.2 GHz cold
    throttle; at K=128 the array is ~91% active, runs at 2.4 GHz, and the
    9x-scaled sum is undone for free by the ScalarE convert's scale=1/9.
  - ScalarE: PSUM fp32 -> SBUF fp16 conversion (x 1/9).
  - VectorE: fp16 running minima (2x perf mode): forward accumulator per
    128-pred block folded by a log2 tensor_tensor tree, backward accumulator
    [128, 16384] across pred blocks.
  - Tail: PE transposes the backward accumulator in 128-col chunks (4 chunks
    per PSUM tile); VectorE reduce_min over [128, 4, 128] -> per-target min.
Host: concatenates forward mins, elementwise-mins backward partials across
quarters, then sqrt + means (O(N) work).
"""

import ml_dtypes
import numpy as np

import concourse.bass as bass
import concourse.tile as tile
from concourse import mybir

F32 = mybir.dt.float32
F16 = mybir.dt.float16
BF16 = mybir.dt.bfloat16

B = 2
N = 16384          # preds per batch
M = 16384          # targets per batch
NQ = N // 4        # preds per core
KA = 13            # base augmented contraction depth
NREP = 9           # replication count (13*9 = 117 <= 128)
K = 128            # padded contraction depth
G = 2048           # PSUM group width (4 banks)
NB = NQ // 128     # pred blocks per core (32)
NG = M // G        # target groups (8)
MM_N = 512         # matmul free dim (one PSUM bank)
N_CORES = 8


# --------------------------------------------------------------------------
# Workaround: this walrus build accepts at most one sync-wait command per
# instruction. Hoist extra waits onto same-engine NoOps placed just before.
# --------------------------------------------------------------------------

def _split_sync_waits(nc):
    counter = 0
    for block in nc.m.functions[0].blocks:
        insts = block.instructions
        out = []
        changed = False
        for inst in insts:
            si = inst.sync_info
            if si is not None and si.on_wait and len(si.on_wait) > 1:
                waits = list(si.on_wait)
                for w in waits[:-1]:
                    counter += 1
                    out.append(
                        mybir.InstNoOp(
                            name=f"waitnop-{counter}",
                            engine=inst.engine,
                            sync_info=mybir.SyncInfo(on_wait=[w], on_update=[]),
                        )
                    )
                si.on_wait = waits[-1:]
                changed = True
            out.append(inst)
        if changed:
            block.instructions = out


def _patch_bass():
    if getattr(bass.Bass, "_split_waits_patched", False):
        return
    orig = bass.Bass.to_json_bytes

    def to_json_bytes(self, *a, **kw):
        _split_sync_waits(self)
        return orig(self, *a, **kw)

    bass.Bass.to_json_bytes = to_json_bytes
    bass.Bass._split_waits_patched = True


# --------------------------------------------------------------------------
# Kernel builder
# --------------------------------------------------------------------------

def build_kernel(n_loop: int = 0):
    """n_loop=0: production straight-line kernel. n_loop>0: wrap the main
    (idempotent) compute in a For_i loop for slope timing."""
    _patch_bass()
    nc = bass.Bass()
    paug_d = nc.dram_tensor("paug", [K, NQ], BF16, kind="ExternalInput")
    taug_d = nc.dram_tensor("taug", [K, M], BF16, kind="ExternalInput")
    fmin_d = nc.dram_tensor("fmin", [128, NB], F32, kind="ExternalOutput")
    bmin_d = nc.dram_tensor("bmin", [128, M // 128], F32, kind="ExternalOutput")

    with tile.TileContext(nc) as tc:
        with (
            tc.tile_pool(name="singles", bufs=1) as singles,
            tc.tile_pool(name="work", bufs=3) as work,
        ):
            paug = singles.tile([K, NQ], BF16)
            taug = singles.tile([K, M], BF16)
            bacc = singles.tile([128, M], F16)
            fmin_sb = singles.tile([128, NB], F32)
            bmin_sb = singles.tile([128, M // 128], F32)

            nc.sync.dma_start(out=paug[:], in_=paug_d[:])
            for g in range(NG):
                nc.sync.dma_start(
                    out=taug[:, g * G:(g + 1) * G],
                    in_=taug_d[:, g * G:(g + 1) * G],
                )

            ident = singles.tile([128, 128], F16)
            nc.gpsimd.memset(ident[:], 0.0)
            nc.gpsimd.affine_select(
                out=ident[:],
                in_=ident[:],
                compare_op=mybir.AluOpType.not_equal,
                fill=1.0,
                base=0,
                pattern=[[-1, 128]],
                channel_multiplier=1,
            )

            CVT_SCALE = 1.0 / NREP
            GP = 2 * G  # paired-group width for DVE ops (4096)

            def main_compute():
                for nb in range(NB):
                    lhsT = paug[:, nb * 128:(nb + 1) * 128]
                    facc = work.tile([128, GP], F16, name=f"facc{nb}",
                                     tag="facc")
                    for gp in range(NG // 2):
                        if nb == 0:
                            # first pred block: convert straight into bacc
                            cvt_pair = bacc[:, gp * GP:(gp + 1) * GP]
                        elif gp == 0:
                            # convert straight into the fresh forward acc
                            cvt_pair = facc[:]
                        else:
                            cvt_t = work.tile([128, GP], F16,
                                              name=f"cvt{nb}_{gp}", tag="cvt")
                            cvt_pair = cvt_t[:]
                        for h in range(2):
                            g = gp * 2 + h
                            d2 = psum.tile([128, G], F32, name=f"d2_{nb}_{g}",
                                           tag="d2")
                            for j in range(G // MM_N):
                                nc.tensor.matmul(
                                    d2[:, j * MM_N:(j + 1) * MM_N],
                                    lhsT,
                                    taug[:, g * G + j * MM_N:
                                         g * G + (j + 1) * MM_N],
                                    start=True,
                                    stop=True,
                                )
                            nc.scalar.activation(
                                out=cvt_pair[:, h * G:(h + 1) * G], in_=d2[:],
                                func=mybir.ActivationFunctionType.Copy,
                                scale=CVT_SCALE,
                            )
                        if nb == 0:
                            # fwd accumulates out of bacc slices
                            if gp == 0:
                                nc.vector.tensor_copy(facc[:], cvt_pair)
                            else:
                                nc.vector.tensor_tensor(
                                    out=facc[:], in0=facc[:], in1=cvt_pair,
                                    op=mybir.AluOpType.min,
                                )
                        else:
                            nc.vector.tensor_tensor(
                                out=bacc[:, gp * GP:(gp + 1) * GP],
                                in0=bacc[:, gp * GP:(gp + 1) * GP],
                                in1=cvt_pair,
                                op=mybir.AluOpType.min,
                            )
                            if gp != 0:
                                nc.vector.tensor_tensor(
                                    out=facc[:], in0=facc[:], in1=cvt_pair,
                                    op=mybir.AluOpType.min,
                                )
                    # 3-level fold of facc -> fmin column (2 TT halvings
                    # at 2x mode, then one 1x reduce_min over 1024)
                    nc.vector.tensor_tensor(
                        out=facc[:, 0:2048], in0=facc[:, 0:2048],
                        in1=facc[:, 2048:4096], op=mybir.AluOpType.min,
                    )
                    nc.vector.tensor_tensor(
                        out=facc[:, 0:1024], in0=facc[:, 0:1024],
                        in1=facc[:, 1024:2048], op=mybir.AluOpType.min,
                    )
                    nc.vector.tensor_reduce(
                        out=fmin_sb[:, nb:nb + 1], in_=facc[:, 0:1024],
                        axis=mybir.AxisListType.X, op=mybir.AluOpType.min,
                    )

            with tc.tile_pool(name="psum", bufs=2, space="PSUM") as psum:
                def tail_fold():
                    # backward partition fold: transpose 4x128 cols per PSUM
                    # tile, one reduce per 4 chunks
                    for t4 in range(M // 512):
                        tp = psum.tile([128, 512], F16, name=f"tp{t4}",
                                       tag="d2")
                        for u in range(4):
                            t = t4 * 4 + u
                            nc.tensor.transpose(
                                tp[:, u * 128:(u + 1) * 128],
                                bacc[:, t * 128:(t + 1) * 128],
                                ident[:],
                            )
                        nc.vector.tensor_reduce(
                            out=bmin_sb[:, t4 * 4:(t4 + 1) * 4],
                            in_=tp[:].rearrange("p (u f) -> p u f", u=4),
                            axis=mybir.AxisListType.X,
                            op=mybir.AluOpType.min,
                        )

                if n_loop:
                    with tc.For_i(0, n_loop, 1):
                        main_compute()
                        tail_fold()
                else:
                    main_compute()
                    tail_fold()

            nc.sync.dma_start(out=fmin_d[:], in_=fmin_sb[:])
            nc.sync.dma_start(out=bmin_d[:], in_=bmin_sb[:])
    return nc


# --------------------------------------------------------------------------
# Host-side prep: augmented coordinate matrices (bf16 hi/lo split), the
# 13-row base replicated NREP times and zero-padded to K=128 rows.
#   k0-2:  pred -2*hi | target hi
#   k3-5:  pred -2*hi | target lo
#   k6-8:  pred -2*lo | target hi
#   k9:    pred 1     | target t2_hi
#   k10:   pred 1     | target t2_lo
#   k11:   pred p2_hi | target 1
#   k12:   pred p2_lo | target 1
# --------------------------------------------------------------------------

def _bf16(x):
    return x.astype(ml_dtypes.bfloat16)


def _aug_parts(coords):
    c = coords.astype(np.float32).T  # [3, n]
    hi = _bf16(c)
    lo = _bf16(c - hi.astype(np.float32))
    n2 = np.sum(c * c, axis=0, dtype=np.float32)
    n2_hi = _bf16(n2)
    n2_lo = _bf16(n2 - n2_hi.astype(np.float32))
    return hi, lo, n2_hi, n2_lo


def _replicate(base):
    out = np.zeros((K, base.shape[1]), dtype=ml_dtypes.bfloat16)
    for r in range(NREP):
        out[r * KA:(r + 1) * KA] = base
    return out


def _aug_pred(coords):
    hi, lo, n2_hi, n2_lo = _aug_parts(coords)
    base = np.zeros((KA, coords.shape[0]), dtype=ml_dtypes.bfloat16)
    m2hi = _bf16(-2.0 * hi.astype(np.float32))
    m2lo = _bf16(-2.0 * lo.astype(np.float32))
    base[0:3] = m2hi
    base[3:6] = m2hi
    base[6:9] = m2lo
    base[9] = 1.0
    base[10] = 1.0
    base[11] = n2_hi
    base[12] = n2_lo
    return _replicate(base)


def _aug_target(coords):
    hi, lo, n2_hi, n2_lo = _aug_parts(coords)
    base = np.zeros((KA, coords.shape[0]), dtype=ml_dtypes.bfloat16)
    base[0:3] = hi
    base[3:6] = lo
    base[6:9] = hi
    base[9] = n2_hi
    base[10] = n2_lo
    base[11] = 1.0
    base[12] = 1.0
    return _replicate(base)


def make_in_maps(pred, target):
    pred = np.asarray(pred, dtype=np.float32)
    target = np.asarray(target, dtype=np.float32)
    in_maps = []
    taugs = [_aug_target(target[b]) for b in range(B)]
    for c in range(N_CORES):
        b, q = divmod(c, 4)
        in_maps.append({
            "paug": _aug_pred(pred[b, q * NQ:(q + 1) * NQ]),
            "taug": taugs[b],
        })
    return in_maps


def postprocess(results):
    total = np.float64(0.0)
    for b in range(B):
        fwd = []
        bwd = None
        for q in range(4):
            r = results[b * 4 + q]
            fwd.append(np.asarray(r["fmin"]).T.reshape(-1))   # n = nb*128+p
            bm = np.asarray(r["bmin"]).T.reshape(-1)          # m = t*128+p
            bwd = bm if bwd is None else np.minimum(bwd, bm)
        fwd = np.concatenate(fwd)
        f = np.sqrt(np.maximum(fwd, 0.0, dtype=np.float32)).mean(dtype=np.float64)
        g = np.sqrt(np.maximum(bwd, 0.0, dtype=np.float32)).mean(dtype=np.float64)
        total += (f + g) / B
    return np.asarray(total, dtype=np.float32)


# --------------------------------------------------------------------------
# PJRT runner (jit built once per process)
# --------------------------------------------------------------------------

def make_runner(nc, n_cores=N_CORES):
    import jax
    from jax.sharding import Mesh, PartitionSpec
    from jax.experimental.shard_map import shard_map
    from concourse.bass2jax import (
        _bass_exec_p,
        install_neuronx_cc_hook,
        partition_id_tensor,
    )

    install_neuronx_cc_hook()
    partition_name = (
        nc.partition_id_tensor.name if nc.partition_id_tensor else None
    )

    in_names, out_names, out_avals, zero_outs = [], [], [], []
    for alloc in nc.m.functions[0].allocations:
        if not isinstance(alloc, mybir.MemoryLocationSet):
            continue
        name = alloc.memorylocations[0].name
        if alloc.kind == "ExternalInput":
            if name != partition_name:
                in_names.append(name)
        elif alloc.kind == "ExternalOutput":
            shape = tuple(alloc.tensor_shape)
            dtype = mybir.dt.np(alloc.dtype)
            out_names.append(name)
            out_avals.append(jax.core.ShapedArray(shape, dtype))
            zero_outs.append(np.zeros(shape, dtype))
    n_params = len(in_names)
    all_in_names = list(in_names) + list(out_names)
    if partition_name is not None:
        all_in_names.append(partition_name)

    def _body(*args):
        operands = list(args)
        if partition_name is not None:
            operands.append(partition_id_tensor())
        outs = _bass_exec_p.bind(
            *operands,
            out_avals=tuple(out_avals),
            in_names=tuple(all_in_names),
            out_names=tuple(out_names),
            lowering_input_output_aliases=(),
            sim_require_finite=True,
            sim_require_nnan=True,
            nc=nc,
        )
        return tuple(outs)

    devices = jax.devices()[:n_cores]
    mesh = Mesh(np.asarray(devices), ("core",))
    in_specs = (PartitionSpec("core"),) * (n_params + len(out_names))
    out_specs = (PartitionSpec("core"),) * len(out_names)
    jitted = jax.jit(
        shard_map(_body, mesh=mesh, in_specs=in_specs, out_specs=out_specs,
                  check_rep=False),
        keep_unused=True,
    )

    dev_cache = {}

    def run(in_maps, cache_key=None):
        import jax as _jax
        from jax.sharding import NamedSharding

        if cache_key is not None and cache_key in dev_cache:
            args = dev_cache[cache_key]
        else:
            concat_in = [
                np.concatenate(
                    [np.asarray(in_maps[c][n]) for c in range(n_cores)], axis=0
                )
                for n in in_names
            ]
            concat_zeros = [
                np.zeros((n_cores * z.shape[0], *z.shape[1:]), z.dtype)
                for z in zero_outs
            ]
            args = concat_in + concat_zeros
            if cache_key is not None:
                sh = NamedSharding(mesh, PartitionSpec("core"))
                args = [_jax.device_put(a, sh) for a in args]
                dev_cache[cache_key] = args
        outs = jitted(*args)
        _jax.block_until_ready(outs)
        return [
            {
                name: np.asarray(outs[i]).reshape(
                    n_cores, *out_avals[i].shape
                )[c]
                for i, name in enumerate(out_names)
            }
            for c in range(n_cores)
        ]

    return run


_CACHE = {}


def kernel(pred, target):
    if "run" not in _CACHE:
        _CACHE["run"] = make_runner(build_kernel(0))
    results = _CACHE["run"](make_in_maps(pred, target))
    return postprocess(results)
